# revision 7
# baseline (speedup 1.0000x reference)
"""Locally-connected conv (per-location weights) + ReLU on 8 Trainium2 cores.

Problem: x (B=64, Cin=64, H=64, W=64), weights (H, W, Cout=64, Cin=64, 3, 3)
  out[r,a,i,j] = relu( sum_{b,c,d} weights[i,j,a,b,c,d] * xpad[r,b,i+c,j+d] )

Sharding: data-parallel over H — core cid owns output rows i in [8*cid, 8*cid+8).
No collectives; pure SPMD with per-core input slices.

Device strategy (per core), v2 — dense K=128/M=128 duals:
  - x planes are packed pairwise into 128-partition tiles A_s = (plane 2s,
    2s+1) x Cin, resident in SBUF for the whole kernel.
  - Vertical-tap pairing: output row pair (2s-1, 2s) consumes tile A_s with
    both planes valid, so one K=128 x M=128 matmul per (j, d) accumulates
    BOTH rows' dual taps (even rows c=0,1; odd rows c=1,2) with a fully
    dense stationary — no zero padding, full PE utilization.
  - Leftover single taps (even rows c=2, odd rows c=0) are K=64 matmuls on
    opposite partition halves / opposite PSUM column groups, so pairs run
    concurrently on disjoint 32x32 PE sub-arrays.
  - Boundary rows 0 and 7 get M=64 duals sharing one PSUM bank.
  - Each output location accumulates its 9 taps in ONE PSUM bank; a single
    ScalarE ReLU (fp32 PSUM -> bf16 SBUF) finishes it — no DVE adds.
  - Weights stream as 8 chunks of 4.7 MB (36 KB contiguous per partition
    line), triple-buffered so the DMA queue never starves; output leaves in
    two 2 MB bf16 transfers (host upcasts to fp32).
"""

import ml_dtypes
import numpy as np

import concourse.bass as bass
import concourse.mybir as mybir
import concourse.tile as tile
from concourse import bacc
from concourse.bass_utils import run_bass_kernel_spmd

B = 64          # batch (= matmul N)
CIN = 64        # in channels
COUT = 64       # out channels
H = 64
W = 64
KS = 3          # conv kernel size
NCORES = 8
RPC = H // NCORES        # output rows per core = 8
NPLANES = RPC + 2        # padded input planes per core = 10
WPAD = W + 2             # 66
NJB = 8                  # j-blocks of 8 columns
CHUNK = 18432            # weight elements per partition per j-block
FP32 = mybir.dt.float32
# bf16 inputs + fp32 PSUM accumulation; bf16 output (host upcasts).
CDT = mybir.dt.bfloat16
NP_CDT = ml_dtypes.bfloat16

_PROGRAM = None
LAST_RESULTS = None


def _d_off(s, jl, d):
    return (((s - 1) * 8 + jl) * 3 + d) * 128


def _b_off(e, jl, d):
    return 9216 + ((e * 8 + jl) * 3 + d) * 64


def _s_off(ri, jl, d):
    return 12288 + ((ri * 8 + jl) * 3 + d) * 64


def _build_program():
    """One Bass program, SPMD across 8 cores (inputs differ per core)."""
    nc = bacc.Bacc("TRN2", target_bir_lowering=False, debug=False,
                   num_devices=NCORES)
    # wt[jb, k(128), CHUNK] — see _pack_weights for the free-dim layout.
    wt = nc.dram_tensor("wt", [NJB, 128, CHUNK], CDT, kind="ExternalInput")
    # xt[plane(10), b, v, r] — padded x planes for this core's rows.
    xt = nc.dram_tensor("xt", [NPLANES, CIN, WPAD, B], CDT,
                        kind="ExternalInput")
    # ot[part(128), half, bank, jb4, jl, r]
    ot = nc.dram_tensor("ot", [128, 2, 4, 4, 8, B], CDT, kind="ExternalOutput")

    with tile.TileContext(nc) as tc:
        with (
            tc.tile_pool(name="xpool", bufs=1) as xpool,
            tc.tile_pool(name="opool", bufs=1) as opool,
            tc.tile_pool(name="wpool", bufs=3) as wpool,
            tc.tile_pool(name="pspool", bufs=2,
                         space=bass.MemorySpace.PSUM) as pspool,
        ):
            # All x planes stay resident: 5 tiles [128=(plane parity, b), v, r].
            A = []
            for s in range(5):
                t = xpool.tile([128, WPAD, B], CDT, tag=f"xp{s}")
                nc.sync.dma_start(
                    t[:], xt[2 * s:2 * s + 2].rearrange("p b v r -> (p b) v r"))
                A.append(t)
            out_sb = opool.tile([128, 2, 4, 4, 8, B], CDT, tag="out")

            for jb in range(NJB):
                wtile = wpool.tile([128, CHUNK], CDT, tag="w")
                nc.sync.dma_start(wtile[:], wt[jb])
                ps = [pspool.tile([128, 8, B], FP32, tag=f"ps{k}",
                                  name=f"ps{k}")
                      for k in range(4)]

                # PSUM has_written clears are partition-masked: start=True
                # only clears the partitions the matmul writes. Track first/
                # last writer per (bank, partition half) so each half of each
                # bank gets exactly one clearing start and one stop.
                calls = []   # (bank, halves_mask, out, lhsT, rhs)
                for jl in range(8):
                    for d in range(KS):
                        v = 8 * jb + jl + d
                        for s in (1, 2, 3):
                            o = _d_off(s, jl, d)
                            calls.append((s, 3, ps[s][:, jl, :],
                                          wtile[:, o:o + 128], A[s][:, v, :]))
                        o = _b_off(0, jl, d)       # row 0 duals -> cols 64-127
                        calls.append((0, 2, ps[0][64:128, jl, :],
                                      wtile[:, o:o + 64], A[0][:, v, :]))
                        o = _b_off(1, jl, d)       # row 7 duals -> cols 0-63
                        calls.append((0, 1, ps[0][0:64, jl, :],
                                      wtile[:, o:o + 64], A[4][:, v, :]))
                        # Alternate even (K rows 0-63) / odd (rows 64-127)
                        # singles so adjacent LDWEIGHTS hit complementary
                        # row groups and overlap on the weight-load path.
                        for il in (0, 1, 2, 3, 4, 5, 6, 7):
                            ri = il // 2
                            o = _s_off(ri, jl, d)
                            if il % 2 == 0:        # even singles: c=2
                                bank = 0 if il == 0 else il // 2
                                sx = il // 2 + 1
                                calls.append((bank, 2, ps[bank][64:128, jl, :],
                                              wtile[0:64, o:o + 64],
                                              A[sx][0:64, v, :]))
                            else:                  # odd singles: c=0
                                bank = 0 if il == 7 else (il + 1) // 2
                                sx = (il - 1) // 2
                                calls.append((bank, 1, ps[bank][0:64, jl, :],
                                              wtile[64:128, o:o + 64],
                                              A[sx][64:128, v, :]))

                first, last = {}, {}
                for idx, (bank, halves, _, _, _) in enumerate(calls):
                    for h in (1, 2):
                        if halves & h:
                            first.setdefault((bank, h), idx)
                            last[(bank, h)] = idx
                firsts, lasts = set(first.values()), set(last.values())
                for idx, (bank, halves, o_ap, l_ap, r_ap) in enumerate(calls):
                    nc.tensor.matmul(o_ap, l_ap, r_ap,
                                     start=(idx in firsts),
                                     stop=(idx in lasts))

                for k in range(4):
                    nc.scalar.activation(
                        out_sb[:, jb // 4, k, jb % 4, :, :], ps[k][:],
                        mybir.ActivationFunctionType.Relu)
                if jb == 3:
                    nc.sync.dma_start(ot[:, 0], out_sb[:, 0])
                if jb == 7:
                    nc.sync.dma_start(ot[:, 1], out_sb[:, 1])
    nc.compile()
    return nc


def _pack_weights(w):
    """weights slice (il 8, j, a, b, c, d) for one core -> [NJB, 128, CHUNK].

    Free-dim layout per partition line (k = vertical-tap parity * 64 + b for
    duals, k = single-parity-specific):
      D [s(3), jl(8), d(3), m(128)]   dual rows (2s-1, 2s); m = half*64 + a
      B [e(2), jl(8), d(3), m(64)]    e=0 row 0 (c=ph), e=1 row 7 (c=ph+1)
      S [ri(4), jl(8), d(3), a(64)]   parts 0-63: even rows c=2;
                                      parts 64-127: odd rows c=0
    """
    lo = w[[1, 3, 5]][:, :, :, :, 1:3, :]   # rows 2s-1, c=ph+1: [s,j,a,b,ph,d]
    hi = w[[2, 4, 6]][:, :, :, :, 0:2, :]   # rows 2s,   c=ph

    def dpart(arr):  # [s, jg, a, b, ph, d] -> [jb, ph, b, s, jl, d, a]
        t = arr.transpose(4, 3, 0, 1, 5, 2)
        t = t.reshape(2, 64, 3, 8, 8, 3, 64)
        return t.transpose(3, 0, 1, 2, 4, 5, 6)

    D = np.stack([dpart(lo), dpart(hi)], axis=6)   # [jb,ph,b,s,jl,d,half,a]
    D = D.reshape(NJB, 128, 9216)

    r0 = w[0][:, :, :, 0:2, :]   # [jg, a, b, ph, d], c=ph
    r7 = w[7][:, :, :, 1:3, :]   # c=ph+1

    def bpart(arr):  # [jg, a, b, ph, d] -> [jb, ph, b, jl, d, a]
        t = arr.transpose(3, 2, 0, 4, 1)
        t = t.reshape(2, 64, 8, 8, 3, 64)
        return t.transpose(2, 0, 1, 3, 4, 5)

    Bv = np.stack([bpart(r0), bpart(r7)], axis=3)  # [jb, ph, b, e, jl, d, a]
    Bv = Bv.reshape(NJB, 128, 3072)

    ev = w[[0, 2, 4, 6]][:, :, :, :, 2, :]   # [row, jg, a, b, d]
    od = w[[1, 3, 5, 7]][:, :, :, :, 0, :]

    def spart(arr):  # [row, jg, a, b, d] -> [jb, b, row, jl, d, a]
        t = arr.transpose(3, 0, 1, 4, 2)
        t = t.reshape(64, 4, 8, 8, 3, 64)
        return t.transpose(2, 0, 1, 3, 4, 5)

    S = np.concatenate([spart(ev), spart(od)], axis=1)  # [jb,128,row,jl,d,a]
    S = S.reshape(NJB, 128, 6144)

    return np.concatenate([D, Bv, S], axis=2)  # [NJB, 128, CHUNK]


def _prep_x(x):
    xpad = np.pad(x, ((0, 0), (0, 0), (1, 1), (1, 1)))
    return np.ascontiguousarray(xpad.transpose(2, 1, 3, 0))  # [u, b, v, r]


_ROWS_LO = (7, 1, 3, 5)   # PSUM parts 0-63 by bank
_ROWS_HI = (0, 2, 4, 6)   # PSUM parts 64-127 by bank


def _unpack_out(ot_core):
    """ot [128, 2, 4, 4, 8, 64] bf16 -> [r, a, il, j] fp32 for one core."""
    view = np.asarray(ot_core, dtype=np.float32).reshape(2, 64, 2, 4, 4, 8, B)
    res = np.empty((B, COUT, RPC, W), np.float32)
    for bank in range(4):
        for ph, row in ((0, _ROWS_LO[bank]), (1, _ROWS_HI[bank])):
            arr = view[ph, :, :, bank]                    # [a, half, jb4, jl, r]
            res[:, :, row, :] = arr.transpose(4, 0, 1, 2, 3).reshape(B, COUT, W)
    return res


def kernel(x, weights):
    global _PROGRAM, LAST_RESULTS
    x = np.ascontiguousarray(np.asarray(x, dtype=np.float32))
    weights = np.ascontiguousarray(np.asarray(weights, dtype=np.float32))
    assert x.shape == (B, CIN, H, W) and weights.shape == (H, W, COUT, CIN, KS, KS)

    x_t = _prep_x(x).astype(NP_CDT)

    in_maps = []
    for cid in range(NCORES):
        wh = _pack_weights(weights[RPC * cid:RPC * cid + RPC]).astype(NP_CDT)
        in_maps.append({
            "wt": np.ascontiguousarray(wh),
            "xt": np.ascontiguousarray(x_t[RPC * cid:RPC * cid + NPLANES]),
        })

    if _PROGRAM is None:
        _PROGRAM = _build_program()
    res = run_bass_kernel_spmd(_PROGRAM, in_maps, list(range(NCORES)))
    LAST_RESULTS = res

    full = np.empty((B, COUT, H, W), np.float32)
    for cid in range(NCORES):
        full[:, :, RPC * cid:RPC * cid + RPC, :] = _unpack_out(
            res.results[cid]["ot"])
    return full


# revision 10
# speedup vs baseline: 1.0419x; 1.0419x over previous
"""Locally-connected conv (per-location weights) + ReLU on 8 Trainium2 cores.

Problem: x (B=64, Cin=64, H=64, W=64), weights (H, W, Cout=64, Cin=64, 3, 3)
  out[r,a,i,j] = relu( sum_{b,c,d} weights[i,j,a,b,c,d] * xpad[r,b,i+c,j+d] )

Sharding: data-parallel over H — core cid owns output rows i in [8*cid, 8*cid+8).
No collectives; pure SPMD with per-core input slices.

Device strategy (per core), v6 — x-stationary / weight-streaming:
  The PE weight-load path (LDWEIGHTS, ~1 column per 1.2 GHz cycle) is the
  bottleneck when the per-location weights are the stationary operand: every
  weight element is used only B=64 times, so weight-stationary schedules pay
  ~2500 LDWEIGHTS streams. Inverting the operands fixes this:
  - stationary = x column slice A_s[:, v, :]  (K=128 plane-pair x Cin,
    M=64 batch), reused by every weight column that consumes (pair s, v);
  - moving    = packed weight blocks (K up to 128, N up to 384), streamed
    from SBUF at ~1 column/cycle warm — each weight element crosses once.
  PSUM holds [batch, 4 j, 2 row, Cout] per bank; v-adjacent matmuls
  accumulate into overlapping j slots (tap offset d maps j = v-d).
  Vertical tap pairs (c, c+1) contract over K=128 plane pairs; leftover
  single taps run as K=64 matmuls, ALL placed on PE rows 64-127 (odd rows
  use A-pair lower halves, even rows use extra odd-parity pair tiles) so
  any two singles hitting the same PSUM partition half share row groups
  and serialize — concurrent same-bank-same-partition PSUM writes from
  disjoint sub-arrays are a fatal HW collision.
  One ScalarE ReLU (fp32 PSUM -> bf16 SBUF) per bank; output leaves in
  four 1 MB bf16 DMAs (host upcasts to fp32).
"""

import ml_dtypes
import numpy as np

import concourse.bass as bass
import concourse.mybir as mybir
import concourse.tile as tile
from concourse import bacc
from concourse.bass_utils import run_bass_kernel_spmd

B = 64          # batch (= matmul M: psum partitions)
CIN = 64        # in channels
COUT = 64       # out channels
H = 64
W = 64
KS = 3          # conv kernel size
NCORES = 8
RPC = H // NCORES        # output rows per core = 8
NPLANES = RPC + 2        # padded input planes per core = 10
WPAD = W + 2             # 66
NG = 16                  # j-groups of 4 columns
CH = 6144                # main weight chunk: per-partition elems per group
CH2 = 6144               # singles chunk (partitions 64-127 only)
NJV = (1, 2, 3, 3, 2, 1)   # j's touched by v = 4g+vi within group g
PREF = (0, 1, 3, 6, 9, 11)
FP32 = mybir.dt.float32
CDT = mybir.dt.bfloat16
NP_CDT = ml_dtypes.bfloat16

_PROGRAM = None
LAST_RESULTS = None


def _m_off(s, vi):
    return (s - 1) * 1536 + PREF[vi] * 128


def _bd_off(e, vi):
    return 4608 + e * 768 + PREF[vi] * 64


def _s2_off(i, vi):
    return i * 768 + PREF[vi] * 64


# pair -> (bank, half): bank0 lo=pair1, hi=pair2; bank1 lo=pair3, hi=boundary
_TGT = {1: (0, 0), 2: (0, 1), 3: (1, 0), 'B': (1, 1)}


def _build_program():
    """One Bass program, SPMD across 8 cores (inputs differ per core)."""
    nc = bacc.Bacc("TRN2", target_bir_lowering=False, debug=False,
                   num_devices=NCORES)
    wt = nc.dram_tensor("wt", [NG, 128, CH], CDT, kind="ExternalInput")
    wt2 = nc.dram_tensor("wt2", [NG, 64, CH2], CDT, kind="ExternalInput")
    # xt[plane(10), b, v, r] — padded x planes for this core's rows.
    xt = nc.dram_tensor("xt", [NPLANES, CIN, WPAD, B], CDT,
                        kind="ExternalInput")
    # ot[quarter, part(128), gq, sslot, jr, rs, a]
    ot = nc.dram_tensor("ot", [4, 128, 4, 2, 4, 2, COUT], CDT,
                        kind="ExternalOutput")

    with tile.TileContext(nc) as tc:
        with (
            tc.tile_pool(name="xpool", bufs=1) as xpool,
            tc.tile_pool(name="opool", bufs=2) as opool,
            tc.tile_pool(name="wpool", bufs=4) as wpool,
            tc.tile_pool(name="pspool", bufs=3,
                         space=bass.MemorySpace.PSUM) as pspool,
        ):
            # x plane-pair tiles, resident all kernel:
            # A_s = planes (2s, 2s+1); Bt_s = planes (2s+1, 2s+2).
            A, Bt = [], []
            for s in range(5):
                t = xpool.tile([128, WPAD, B], CDT, tag=f"xp{s}")
                nc.sync.dma_start(
                    t[:], xt[2 * s:2 * s + 2].rearrange("p b v r -> (p b) v r"))
                A.append(t)
            for s in range(4):
                t = xpool.tile([128, WPAD, B], CDT, tag=f"xq{s}")
                nc.sync.dma_start(
                    t[:],
                    xt[2 * s + 1:2 * s + 3].rearrange("p b v r -> (p b) v r"))
                Bt.append(t)

            out_q = None
            for g in range(NG):
                if g % 4 == 0:
                    out_q = opool.tile([128, 4, 2, 4, 2, COUT], CDT, tag="oq")
                gq = g % 4
                wtile = wpool.tile([128, CH], CDT, tag="w")
                nc.sync.dma_start(wtile[:], wt[g])
                wtile2 = wpool.tile([128, CH2], CDT, tag="w2")
                nc.sync.dma_start(wtile2[64:128, :], wt2[g])
                bk = [pspool.tile([128, 4, 2, COUT], FP32, tag=f"bk{k}",
                                  name=f"bk{k}")
                      for k in range(2)]

                # (bank, half, out_ap, lhsT, rhs)
                calls = []
                for vi in range(6):
                    v = 4 * g + vi
                    j0 = max(v - 2, 4 * g)
                    nj = NJV[vi]
                    jr0 = j0 - 4 * g
                    for s in (1, 2, 3):       # dual-tap row pairs
                        b_, h_ = _TGT[s]
                        o = _m_off(s, vi)
                        calls.append((
                            b_, h_,
                            bk[b_][h_ * 64:h_ * 64 + 64, jr0:jr0 + nj, :, :],
                            A[s][:, v, :],
                            wtile[:, o:o + nj * 128]))
                    # boundary duals: e=0 row 0 -> rs1; e=1 row 7 -> rs0
                    for e, sA, rs in ((0, 0, 1), (1, 4, 0)):
                        b_, h_ = _TGT['B']
                        o = _bd_off(e, vi)
                        calls.append((
                            b_, h_,
                            bk[b_][h_ * 64:h_ * 64 + 64, jr0:jr0 + nj, rs, :],
                            A[sA][:, v, :],
                            wtile[:, o:o + nj * 64]))
                    # single taps, all on PE rows 64-127:
                    # even row i: c=2, plane i+2 = lower half of Bt[i//2]
                    # odd row i:  c=0, plane i   = lower half of A[(i-1)//2]
                    for i in range(8):
                        sp = i // 2 if i % 2 == 0 else (i + 1) // 2
                        rs = 1 if i % 2 == 0 else 0
                        b_, h_ = _TGT['B' if sp in (0, 4) else sp]
                        o = _s2_off(i, vi)
                        xtile = Bt[i // 2] if i % 2 == 0 else A[(i - 1) // 2]
                        calls.append((
                            b_, h_,
                            bk[b_][h_ * 64:h_ * 64 + 64, jr0:jr0 + nj, rs, :],
                            xtile[64:128, v, :],
                            wtile2[64:128, o:o + nj * 64]))

                # PSUM has_written clears are partition-masked; every
                # (bank, half) needs exactly one clearing start + one stop.
                first, last = {}, {}
                for idx_, (b_, h_, *_r) in enumerate(calls):
                    first.setdefault((b_, h_), idx_)
                    last[(b_, h_)] = idx_
                firsts, lasts = set(first.values()), set(last.values())
                for idx_, (b_, h_, o_ap, l_ap, r_ap) in enumerate(calls):
                    nc.tensor.matmul(o_ap, l_ap, r_ap,
                                     start=(idx_ in firsts),
                                     stop=(idx_ in lasts))

                # ReLU psum -> bf16 out slice, one full-bank ACT per bank
                # (a half-bank ACT could read while the PE still writes the
                # other half of the same bank — a fatal PSUM collision).
                for ss in range(2):
                    nc.scalar.activation(
                        out_q[:, gq, ss], bk[ss][:],
                        mybir.ActivationFunctionType.Relu)
                if gq == 3:
                    nc.sync.dma_start(ot[g // 4], out_q[:])
    nc.compile()
    return nc


def _pack_weights(w):
    """weights slice (il 8, j, a, b, c, d) for one core -> (wt, wt2).

    wt [NG, 128, CH], partition k = ph*64 + b:
      M  [s(3), vi-blocks] dual row pairs (2s-1, 2s): [j, rowsel, a] cols,
         c = ph + 1 - rowsel
      Bd [e(2), vi-blocks] boundary: e0 row 0 (c=ph, rs1), e1 row 7 (c=ph+1)
    wt2 [NG, 64, CH2], partition b (lands on PE rows 64-127):
      S  [row i(8), vi-blocks] single taps: even i c=2, odd i c=0
    """
    wt = np.zeros((NG, 128, CH), np.float32)
    wt2 = np.zeros((NG, 64, CH2), np.float32)
    wT = w.transpose(0, 4, 5, 1, 3, 2)  # [i, c, d, j, b, a]
    for g in range(NG):
        for vi in range(6):
            v = 4 * g + vi
            j0 = max(v - 2, 4 * g)
            nj = NJV[vi]
            js = range(j0, j0 + nj)
            for s in (1, 2, 3):
                blk = np.empty((2, 64, nj, 2, 64), np.float32)
                for ph in range(2):
                    for rs in range(2):
                        i, c = 2 * s - 1 + rs, ph + 1 - rs
                        for ji, j in enumerate(js):
                            blk[ph, :, ji, rs, :] = wT[i, c, v - j, j]
                o = _m_off(s, vi)
                wt[g, :, o:o + nj * 128] = blk.reshape(128, -1)
            for e, i in ((0, 0), (1, 7)):
                blk = np.empty((2, 64, nj, 64), np.float32)
                for ph in range(2):
                    c = ph + e
                    for ji, j in enumerate(js):
                        blk[ph, :, ji, :] = wT[i, c, v - j, j]
                o = _bd_off(e, vi)
                wt[g, :, o:o + nj * 64] = blk.reshape(128, -1)
            for i in range(8):
                c = 2 if i % 2 == 0 else 0
                blk = np.empty((64, nj, 64), np.float32)
                for ji, j in enumerate(js):
                    blk[:, ji, :] = wT[i, c, v - j, j]
                o = _s2_off(i, vi)
                wt2[g, :, o:o + nj * 64] = blk.reshape(64, -1)
    return wt, wt2


def _prep_x(x):
    xpad = np.pad(x, ((0, 0), (0, 0), (1, 1), (1, 1)))
    return np.ascontiguousarray(xpad.transpose(2, 1, 3, 0))  # [u, b, v, r]


# (half, sslot) -> output rows (rs0, rs1)
_ROWMAP = {(0, 0): (1, 2), (0, 1): (5, 6), (1, 0): (3, 4), (1, 1): (7, 0)}


def _unpack_out(ot_core):
    """ot [4, 128, 4, 2, 4, 2, 64] bf16 -> [r, a, il, j] fp32 for one core."""
    v = np.asarray(ot_core, dtype=np.float32)
    # -> [part, g(16), sslot, jr, rs, a] with g = q*4 + gq
    v = v.transpose(1, 0, 2, 3, 4, 5, 6).reshape(128, NG, 2, 4, 2, COUT)
    res = np.empty((B, COUT, RPC, W), np.float32)
    for hf in range(2):
        for ss in range(2):
            r0, r1 = _ROWMAP[(hf, ss)]
            arr = v[hf * 64:hf * 64 + 64, :, ss]       # [r, g, jr, rs, a]
            for rs, row in ((0, r0), (1, r1)):
                a_ = arr[:, :, :, rs, :].reshape(B, W, COUT)  # [r, j, a]
                res[:, :, row, :] = a_.transpose(0, 2, 1)
    return res


def kernel(x, weights):
    global _PROGRAM, LAST_RESULTS
    x = np.ascontiguousarray(np.asarray(x, dtype=np.float32))
    weights = np.ascontiguousarray(np.asarray(weights, dtype=np.float32))
    assert x.shape == (B, CIN, H, W) and weights.shape == (H, W, COUT, CIN, KS, KS)

    x_t = _prep_x(x).astype(NP_CDT)

    in_maps = []
    for cid in range(NCORES):
        wh, wh2 = _pack_weights(weights[RPC * cid:RPC * cid + RPC])
        in_maps.append({
            "wt": np.ascontiguousarray(wh.astype(NP_CDT)),
            "wt2": np.ascontiguousarray(wh2.astype(NP_CDT)),
            "xt": np.ascontiguousarray(x_t[RPC * cid:RPC * cid + NPLANES]),
        })

    if _PROGRAM is None:
        _PROGRAM = _build_program()
    res = run_bass_kernel_spmd(_PROGRAM, in_maps, list(range(NCORES)))
    LAST_RESULTS = res

    full = np.empty((B, COUT, H, W), np.float32)
    for cid in range(NCORES):
        full[:, :, RPC * cid:RPC * cid + RPC, :] = _unpack_out(
            res.results[cid]["ot"])
    return full
